# revision 12
# baseline (speedup 1.0000x reference)
"""Chunk-based multi-head attention TRN2 kernel (8-core SPMD).

Full model: x[S,B,E] -> in_proj -> 16-head attention with block-causal
64-chunk mask -> out_proj.  Sharding: B(2) x head-groups(4) over 8 cores;
each core computes 4 heads of one batch and a partial out_proj, reduced
on host.

Key layout trick: scores are computed transposed (scoresT[t,s]) so no
on-device transposes are needed anywhere.  The softmax denominator is
produced by a ones-stationary matmul (partition-aligned with the PV
output), so normalization is a plain elementwise reciprocal+mul.
The chunk mask (masked iff chunk(t) > chunk(s)) is block-causal: key
chunk T contributes only to query columns s >= (2T-8c)*64 within a
512-query chunk c, plus one 64x64 memset for the half-chunk staircase.
"""

import sys

if "/opt/trn_rl_repo" not in sys.path:
    sys.path.insert(0, "/opt/trn_rl_repo")

import numpy as np

import concourse.bass as bass
import concourse.mybir as mybir
import concourse.tile as tile
from concourse import bacc, bass_utils

S = 2048          # sequence length
B = 2             # batch
E = 1024          # embed dim
H = 16            # total heads
HL = 4            # heads per core
D = 64            # head dim
FQK = 2 * HL * D  # local q+k features = 512
FV = HL * D       # local v features = 256
KT = E // 128     # 8 contraction chunks for projections
NC = S // 512     # 4 query 512-chunks
TT = S // 128     # 16 key 128-chunks
N_CORES = 8

F32 = mybir.dt.float32
F32R = mybir.dt.float32r

USE_F32R = True   # float32r: 4x faster PE, slightly relaxed precision
TRACE = False     # set by test.py for profiling runs
LAST_RESULT = None

_NC_CACHE = {}


def _dt():
    return F32R if USE_F32R else F32


def _mm(ap):
    return ap


def _body(nc, tc, xT_d, wqkT_d, wvT_d, bqk_d, bv_d, woT_d, y_d):
    from contextlib import ExitStack

    ctx = ExitStack()
    with ctx:
        P = ctx.enter_context(tc.tile_pool(name="persist", bufs=1))
        xp = ctx.enter_context(tc.tile_pool(name="xstream", bufs=2))
        ep = ctx.enter_context(tc.tile_pool(name="etiles", bufs=1))
        yp = ctx.enter_context(tc.tile_pool(name="ytiles", bufs=1))

        # ---- persistent SBUF ----
        qkT = P.tile([128, 4, S], _dt())        # [p, m, s]; m0/1 = qT pairs, m2/3 = kT pairs
        outT = P.tile([128, 2, S], _dt())       # [dl%128, kk, s] normalized attn out, transposed
        vaug = P.tile([128, TT, HL, 65], _dt())  # per (T,h): [V_h(64) | ones]
        wqk_sb = P.tile([128, KT, FQK], _dt())
        wv_sb = P.tile([128, KT, FV], _dt())
        wo_sb = P.tile([128, 2, E], _dt())
        bqk_sb = P.tile([128, 4], F32)
        bv_sb = P.tile([1, FV], _dt())
        ones_row = P.tile([1, 128], _dt())      # K=1 stationary for v bias
        ones_p64 = P.tile([128, 64], _dt())     # K=1 stationary at partition 64 (recip bcast)

        for k in range(KT):
            nc.sync.dma_start(out=wqk_sb[:, k, :], in_=wqkT_d[k * 128:(k + 1) * 128, :])
            nc.sync.dma_start(out=wv_sb[:, k, :], in_=wvT_d[k * 128:(k + 1) * 128, :])
        for kk in range(2):
            nc.sync.dma_start(out=wo_sb[:, kk, :], in_=woT_d[kk * 128:(kk + 1) * 128, :])
        nc.sync.dma_start(out=bqk_sb, in_=bqk_d.rearrange("(m p) -> p m", p=128))
        nc.sync.dma_start(out=bv_sb, in_=bv_d)
        nc.vector.memset(ones_row.bitcast(F32), 1.0)
        nc.vector.memset(ones_p64.bitcast(F32), 1.0)
        nc.vector.memset(vaug[:, :, :, 64:65].bitcast(F32), 1.0)

        # ---- in_proj: qkT[f, s] = Wqk @ x.T ; v[s, d] = x @ Wv.T ----
        psA = tc.alloc_tile_pool(name="psum_in", bufs=1, space="PSUM")
        for n in range(NC):
            xq = xp.tile([128, KT, 512], _dt(), tag="xq")
            for k in range(KT):
                nc.sync.dma_start(out=xq[:, k, :], in_=xT_d[k * 128:(k + 1) * 128, n * 512:(n + 1) * 512])
            for m in range(4):
                ps_qk = psA.tile([128, 512], F32, tag="qk", bufs=2)
                for k in range(KT):
                    nc.tensor.matmul(
                        ps_qk,
                        _mm(wqk_sb[:, k, m * 128:(m + 1) * 128]),
                        _mm(xq[:, k, :]),
                        start=(k == 0), stop=(k == KT - 1),
                    )
                # bias is per-partition (feature) here -> fused into copy-out
                nc.vector.tensor_scalar_add(qkT[:, m, n * 512:(n + 1) * 512], ps_qk, bqk_sb[:, m:m + 1])
            for tt in range(4):
                t = 4 * n + tt
                ps_v = psA.tile([128, FV], F32, tag="v", bufs=2)
                for k in range(KT):
                    nc.tensor.matmul(
                        ps_v,
                        _mm(xq[:, k, tt * 128:(tt + 1) * 128]),
                        _mm(wv_sb[:, k, :]),
                        start=(k == 0), stop=False,
                    )
                # + bias via ones-row outer product
                nc.tensor.matmul(ps_v, _mm(ones_row), _mm(bv_sb), start=False, stop=True)
                nc.vector.tensor_copy(vaug[:, t, :, 0:64], ps_v.rearrange("p (h d) -> p h d", h=HL))
        psA.release()

        # ---- attention, per head h ----
        # pv psum rows 0-63 = unnormalized attn-out (transposed), row 64 = softmax denom.
        # recip(denom) row is broadcast to partitions 0-63 via a K=1 ones matmul;
        # normalization is then a partition-aligned elementwise multiply.
        psB = ctx.enter_context(tc.tile_pool(name="psum_at", bufs=1, space="PSUM"))
        for h in range(HL):
            kk = h // 2
            po = (h % 2) * 64
            for c in range(NC):
                pv = psB.tile([65, 512], F32, tag="pv", bufs=2)
                t_max = 4 * c + 3
                for T in range(t_max + 1):
                    s0 = max(0, (2 * T - 8 * c) * 64)
                    sc = psB.tile([128, 512], F32, tag="sc", bufs=4)
                    nc.tensor.matmul(
                        sc[:, s0:512],
                        _mm(qkT[po:po + 64, 2 + kk, T * 128:(T + 1) * 128]),
                        _mm(qkT[po:po + 64, kk, c * 512 + s0:(c + 1) * 512]),
                        start=True, stop=True,
                    )
                    e_t = ep.tile([128, 512], _dt(), tag="e", bufs=4)
                    nc.scalar.activation(
                        e_t[:, s0:512], sc[:, s0:512],
                        mybir.ActivationFunctionType.Exp, scale=0.125,
                    )
                    if 2 * T - 8 * c >= 0:
                        # staircase: key chunk 2T+1 masked for first 64 query cols
                        nc.vector.memset(e_t[64:128, s0:s0 + 64].bitcast(F32), 0.0)
                    nc.tensor.matmul(
                        pv[:, s0:512],
                        _mm(vaug[:, T, h, :]),
                        _mm(e_t[:, s0:512]),
                        start=(T == 0), stop=(T == t_max),
                    )
                # denom recip row (partition 64), rounded to f32r for the bcast matmul
                rrow_f = ep.tile([65, 512], F32, tag="rrf", bufs=2)
                rrow = ep.tile([65, 512], _dt(), tag="rr", bufs=2)
                nc.vector.reciprocal(rrow_f[64:65, :], pv[64:65, :])
                nc.vector.tensor_copy(rrow[64:65, :], rrow_f[64:65, :])
                rc_bc = psB.tile([64, 512], F32, tag="rcb", bufs=2)
                nc.tensor.matmul(rc_bc, _mm(ones_p64[64:65, 0:64]), _mm(rrow[64:65, :]), start=True, stop=True)
                rc_sb = ep.tile([64, 512], F32, tag="rcs", bufs=2)
                nc.vector.tensor_copy(rc_sb, rc_bc)
                if po == 0:
                    nc.vector.tensor_mul(outT[0:64, kk, c * 512:(c + 1) * 512], pv[0:64, :], rc_sb)
                else:
                    stage = ep.tile([64, 512], _dt(), tag="stg", bufs=2)
                    nc.vector.tensor_mul(stage, pv[0:64, :], rc_sb)
                    nc.sync.dma_start(out=outT[64:128, kk, c * 512:(c + 1) * 512], in_=stage)

        # ---- out_proj partial: y[s, e] = outT.T @ WoT ----
        for t in range(TT):
            for n in range(2):
                ps_y = psB.tile([128, 512], F32, tag="sc", bufs=4)
                for kk in range(2):
                    nc.tensor.matmul(
                        ps_y,
                        _mm(outT[:, kk, t * 128:(t + 1) * 128]),
                        _mm(wo_sb[:, kk, n * 512:(n + 1) * 512]),
                        start=(kk == 0), stop=(kk == 1),
                    )
                y_sb = yp.tile([128, 512], F32, tag="ysb", bufs=4)
                if n == 0:
                    nc.vector.tensor_copy(y_sb, ps_y)
                else:
                    nc.scalar.copy(y_sb, ps_y)
                nc.sync.dma_start(out=y_d[t * 128:(t + 1) * 128, n * 512:(n + 1) * 512], in_=y_sb)


def build_program():
    key = ("prog", USE_F32R)
    if key in _NC_CACHE:
        return _NC_CACHE[key]
    nc = bacc.Bacc(
        "TRN2",
        target_bir_lowering=False,
        debug=False,
        enable_asserts=False,
        num_devices=N_CORES,
    )
    xT_d = nc.dram_tensor("xT", [E, S], _dt(), kind="ExternalInput").ap()
    wqkT_d = nc.dram_tensor("wqkT", [E, FQK], _dt(), kind="ExternalInput").ap()
    wvT_d = nc.dram_tensor("wvT", [E, FV], _dt(), kind="ExternalInput").ap()
    bqk_d = nc.dram_tensor("bqk", [FQK], F32, kind="ExternalInput").ap()
    bv_d = nc.dram_tensor("bv", [1, FV], _dt(), kind="ExternalInput").ap()
    woT_d = nc.dram_tensor("woT", [FV, E], _dt(), kind="ExternalInput").ap()
    y_d = nc.dram_tensor("y", [S, E], F32, kind="ExternalOutput").ap()

    with tile.TileContext(nc) as tc:
        _body(nc, tc, xT_d, wqkT_d, wvT_d, bqk_d, bv_d, woT_d, y_d)
    nc.compile()
    _NC_CACHE[key] = nc
    return nc


def make_in_maps(x, in_proj_w, in_proj_b, out_proj_w):
    x = np.asarray(x, dtype=np.float32)
    W = np.asarray(in_proj_w, dtype=np.float32)
    bi = np.asarray(in_proj_b, dtype=np.float32)
    Wo = np.asarray(out_proj_w, dtype=np.float32)
    in_maps = []
    for core in range(N_CORES):
        b = core // 4
        g = core % 4
        qs = slice(g * FV, (g + 1) * FV)
        ks = slice(E + g * FV, E + (g + 1) * FV)
        vs = slice(2 * E + g * FV, 2 * E + (g + 1) * FV)
        in_maps.append({
            "xT": np.ascontiguousarray(x[:, b, :].T),
            "wqkT": np.ascontiguousarray(np.concatenate([W[qs], W[ks]], axis=0).T),
            "wvT": np.ascontiguousarray(W[vs].T),
            "bqk": np.ascontiguousarray(np.concatenate([bi[qs], bi[ks]])),
            "bv": np.ascontiguousarray(bi[vs].reshape(1, FV)),
            "woT": np.ascontiguousarray(Wo[:, g * FV:(g + 1) * FV].T),
        })
    return in_maps


def kernel(x, in_proj_w, in_proj_b, out_proj_w, out_proj_b):
    global LAST_RESULT
    nc = build_program()
    in_maps = make_in_maps(x, in_proj_w, in_proj_b, out_proj_w)
    res = bass_utils.run_bass_kernel_spmd(
        nc, in_maps, core_ids=list(range(N_CORES)), trace=TRACE,
    )
    LAST_RESULT = res
    bo = np.asarray(out_proj_b, dtype=np.float32)
    out = np.zeros((S, B, E), dtype=np.float32)
    for b in range(B):
        acc = res.results[b * 4]["y"].astype(np.float32)
        for g in range(1, 4):
            acc = acc + res.results[b * 4 + g]["y"]
        out[:, b, :] = acc + bo[None, :]
    return out


# revision 13
# speedup vs baseline: 1.0408x; 1.0408x over previous
"""Chunk-based multi-head attention TRN2 kernel (8-core SPMD).

Full model: x[S,B,E] -> in_proj -> 16-head attention with block-causal
64-chunk mask -> out_proj.  Sharding: B(2) x head-groups(4) over 8 cores;
each core computes 4 heads of one batch and a partial out_proj, reduced
on host.

Key layout trick: scores are computed transposed (scoresT[t,s]) so no
on-device transposes are needed anywhere.  The softmax denominator is
produced by a ones-stationary matmul (partition-aligned with the PV
output), so normalization is a plain elementwise reciprocal+mul.
The chunk mask (masked iff chunk(t) > chunk(s)) is block-causal: key
chunk T contributes only to query columns s >= (2T-8c)*64 within a
512-query chunk c, plus one 64x64 memset for the half-chunk staircase.
"""

import sys

if "/opt/trn_rl_repo" not in sys.path:
    sys.path.insert(0, "/opt/trn_rl_repo")

import numpy as np

import concourse.bass as bass
import concourse.mybir as mybir
import concourse.tile as tile
from concourse import bacc, bass_utils

S = 2048          # sequence length
B = 2             # batch
E = 1024          # embed dim
H = 16            # total heads
HL = 4            # heads per core
D = 64            # head dim
FQK = 2 * HL * D  # local q+k features = 512
FV = HL * D       # local v features = 256
KT = E // 128     # 8 contraction chunks for projections
NC = S // 512     # 4 query 512-chunks
TT = S // 128     # 16 key 128-chunks
N_CORES = 8

F32 = mybir.dt.float32
F32R = mybir.dt.float32r

USE_F32R = True   # float32r: 4x faster PE, slightly relaxed precision
TRACE = False     # set by test.py for profiling runs
LAST_RESULT = None

_NC_CACHE = {}


def _dt():
    return F32R if USE_F32R else F32


def _mm(ap):
    return ap


def _body(nc, tc, xT_d, wqkT_d, wvT_d, bqk_d, bv_d, woT_d, y_d):
    from contextlib import ExitStack

    ctx = ExitStack()
    with ctx:
        P = ctx.enter_context(tc.tile_pool(name="persist", bufs=1))
        xp = ctx.enter_context(tc.tile_pool(name="xstream", bufs=2))
        ep = ctx.enter_context(tc.tile_pool(name="etiles", bufs=1))
        yp = ctx.enter_context(tc.tile_pool(name="ytiles", bufs=1))

        # ---- persistent SBUF ----
        qkT = P.tile([128, 4, S], _dt())        # [p, m, s]; m0/1 = qT pairs, m2/3 = kT pairs
        outT = P.tile([128, 2, S], _dt())       # [dl%128, kk, s] normalized attn out, transposed
        vaug = P.tile([128, TT, HL, 65], _dt())  # per (T,h): [V_h(64) | ones]
        wqk_sb = P.tile([128, KT, FQK], _dt())
        wv_sb = P.tile([128, KT, FV], _dt())
        wo_sb = P.tile([128, 2, E], _dt())
        bqk_sb = P.tile([128, 4], F32)
        bv_sb = P.tile([1, FV], _dt())
        ones_row = P.tile([1, 128], _dt())      # K=1 stationary for v bias

        for k in range(KT):
            nc.sync.dma_start(out=wqk_sb[:, k, :], in_=wqkT_d[k * 128:(k + 1) * 128, :])
            nc.sync.dma_start(out=wv_sb[:, k, :], in_=wvT_d[k * 128:(k + 1) * 128, :])
        for kk in range(2):
            nc.sync.dma_start(out=wo_sb[:, kk, :], in_=woT_d[kk * 128:(kk + 1) * 128, :])
        nc.sync.dma_start(out=bqk_sb, in_=bqk_d.rearrange("(m p) -> p m", p=128))
        nc.sync.dma_start(out=bv_sb, in_=bv_d)
        nc.vector.memset(ones_row.bitcast(F32), 1.0)
        nc.vector.memset(vaug[:, :, :, 64:65].bitcast(F32), 1.0)

        # ---- in_proj: qkT[f, s] = Wqk @ x.T ; v[s, d] = x @ Wv.T ----
        psA = tc.alloc_tile_pool(name="psum_in", bufs=1, space="PSUM")
        for n in range(NC):
            xq = xp.tile([128, KT, 512], _dt(), tag="xq")
            for k in range(KT):
                nc.sync.dma_start(out=xq[:, k, :], in_=xT_d[k * 128:(k + 1) * 128, n * 512:(n + 1) * 512])
            for m in range(4):
                ps_qk = psA.tile([128, 512], F32, tag="qk", bufs=2)
                for k in range(KT):
                    nc.tensor.matmul(
                        ps_qk,
                        _mm(wqk_sb[:, k, m * 128:(m + 1) * 128]),
                        _mm(xq[:, k, :]),
                        start=(k == 0), stop=(k == KT - 1),
                    )
                # bias is per-partition (feature) here -> fused into copy-out
                nc.vector.tensor_scalar_add(qkT[:, m, n * 512:(n + 1) * 512], ps_qk, bqk_sb[:, m:m + 1])
            for tt in range(4):
                t = 4 * n + tt
                ps_v = psA.tile([128, FV], F32, tag="v", bufs=2)
                for k in range(KT):
                    nc.tensor.matmul(
                        ps_v,
                        _mm(xq[:, k, tt * 128:(tt + 1) * 128]),
                        _mm(wv_sb[:, k, :]),
                        start=(k == 0), stop=False,
                    )
                # + bias via ones-row outer product
                nc.tensor.matmul(ps_v, _mm(ones_row), _mm(bv_sb), start=False, stop=True)
                nc.vector.tensor_copy(vaug[:, t, :, 0:64], ps_v.rearrange("p (h d) -> p h d", h=HL))
        psA.release()

        # ---- attention, per head h ----
        # pv psum rows 0-63 = unnormalized attn-out (transposed), row 64 = denom.
        # Normalization is kept entirely OFF the PE critical path: raw PV and
        # denom rows are copied to SBUF during the loops; per head the 4 denom
        # rows are gathered to partitions 0-3 (shift DMA), reciprocal'd in one
        # batched DVE op, bounced through DRAM to broadcast across partitions
        # (stride-0 DMA), and applied with an aligned elementwise multiply.
        psB = ctx.enter_context(tc.tile_pool(name="psum_at", bufs=1, space="PSUM"))
        dp = ctx.enter_context(tc.tile_pool(name="dscratch", bufs=1, space="DRAM"))
        drec_dram = dp.tile([HL, NC, 512], F32)
        for h in range(HL):
            kk = h // 2
            po = (h % 2) * 64
            praw = ep.tile([64, NC, 512], F32, tag="praw", bufs=2)
            dall = ep.tile([65, NC, 512], F32, tag="dall", bufs=2)
            den4 = ep.tile([NC, 512], F32, tag="den4", bufs=2)
            drec4 = ep.tile([NC, 512], F32, tag="drec4", bufs=2)
            for c in range(NC):
                pv = psB.tile([65, 512], F32, tag="pv", bufs=2)
                t_max = 4 * c + 3
                for T in range(t_max + 1):
                    s0 = max(0, (2 * T - 8 * c) * 64)
                    sc = psB.tile([128, 512], F32, tag="sc", bufs=4)
                    nc.tensor.matmul(
                        sc[:, s0:512],
                        _mm(qkT[po:po + 64, 2 + kk, T * 128:(T + 1) * 128]),
                        _mm(qkT[po:po + 64, kk, c * 512 + s0:(c + 1) * 512]),
                        start=True, stop=True,
                    )
                    e_t = ep.tile([128, 512], _dt(), tag="e", bufs=4)
                    nc.scalar.activation(
                        e_t[:, s0:512], sc[:, s0:512],
                        mybir.ActivationFunctionType.Exp, scale=0.125,
                    )
                    if 2 * T - 8 * c >= 0:
                        # staircase: key chunk 2T+1 masked for first 64 query cols
                        nc.vector.memset(e_t[64:128, s0:s0 + 64].bitcast(F32), 0.0)
                    nc.tensor.matmul(
                        pv[:, s0:512],
                        _mm(vaug[:, T, h, :]),
                        _mm(e_t[:, s0:512]),
                        start=(T == 0), stop=(T == t_max),
                    )
                nc.vector.tensor_copy(praw[:, c, :], pv[0:64, :])
                nc.scalar.copy(dall[64:65, c, :], pv[64:65, :])
                # shift denom row to partition c for the batched reciprocal
                nc.sync.dma_start(out=den4[c:c + 1, :], in_=dall[64:65, c, :])
            nc.vector.reciprocal(drec4, den4)
            nc.sync.dma_start(out=drec_dram[h], in_=drec4)
            for c in range(NC):
                rsrc = drec_dram[h, c]
                bc_ap = bass.AP(tensor=rsrc.tensor, offset=rsrc.offset,
                                ap=[[0, 64]] + [list(x) for x in rsrc.ap])
                rbc = ep.tile([64, 512], F32, tag="rbc", bufs=4)
                nc.gpsimd.dma_start(out=rbc, in_=bc_ap)
                if po == 0:
                    nc.vector.tensor_mul(outT[0:64, kk, c * 512:(c + 1) * 512], praw[:, c, :], rbc)
                else:
                    stage = ep.tile([64, 512], _dt(), tag="stg", bufs=2)
                    nc.vector.tensor_mul(stage, praw[:, c, :], rbc)
                    nc.sync.dma_start(out=outT[64:128, kk, c * 512:(c + 1) * 512], in_=stage)

        # ---- out_proj partial: y[s, e] = outT.T @ WoT ----
        for t in range(TT):
            for n in range(2):
                ps_y = psB.tile([128, 512], F32, tag="sc", bufs=4)
                for kk in range(2):
                    nc.tensor.matmul(
                        ps_y,
                        _mm(outT[:, kk, t * 128:(t + 1) * 128]),
                        _mm(wo_sb[:, kk, n * 512:(n + 1) * 512]),
                        start=(kk == 0), stop=(kk == 1),
                    )
                y_sb = yp.tile([128, 512], F32, tag="ysb", bufs=4)
                if n == 0:
                    nc.vector.tensor_copy(y_sb, ps_y)
                else:
                    nc.scalar.copy(y_sb, ps_y)
                nc.sync.dma_start(out=y_d[t * 128:(t + 1) * 128, n * 512:(n + 1) * 512], in_=y_sb)


def build_program():
    key = ("prog", USE_F32R)
    if key in _NC_CACHE:
        return _NC_CACHE[key]
    nc = bacc.Bacc(
        "TRN2",
        target_bir_lowering=False,
        debug=False,
        enable_asserts=False,
        num_devices=N_CORES,
    )
    xT_d = nc.dram_tensor("xT", [E, S], _dt(), kind="ExternalInput").ap()
    wqkT_d = nc.dram_tensor("wqkT", [E, FQK], _dt(), kind="ExternalInput").ap()
    wvT_d = nc.dram_tensor("wvT", [E, FV], _dt(), kind="ExternalInput").ap()
    bqk_d = nc.dram_tensor("bqk", [FQK], F32, kind="ExternalInput").ap()
    bv_d = nc.dram_tensor("bv", [1, FV], _dt(), kind="ExternalInput").ap()
    woT_d = nc.dram_tensor("woT", [FV, E], _dt(), kind="ExternalInput").ap()
    y_d = nc.dram_tensor("y", [S, E], F32, kind="ExternalOutput").ap()

    with tile.TileContext(nc) as tc:
        _body(nc, tc, xT_d, wqkT_d, wvT_d, bqk_d, bv_d, woT_d, y_d)
    nc.compile()
    _NC_CACHE[key] = nc
    return nc


def make_in_maps(x, in_proj_w, in_proj_b, out_proj_w):
    x = np.asarray(x, dtype=np.float32)
    W = np.asarray(in_proj_w, dtype=np.float32)
    bi = np.asarray(in_proj_b, dtype=np.float32)
    Wo = np.asarray(out_proj_w, dtype=np.float32)
    in_maps = []
    for core in range(N_CORES):
        b = core // 4
        g = core % 4
        qs = slice(g * FV, (g + 1) * FV)
        ks = slice(E + g * FV, E + (g + 1) * FV)
        vs = slice(2 * E + g * FV, 2 * E + (g + 1) * FV)
        in_maps.append({
            "xT": np.ascontiguousarray(x[:, b, :].T),
            "wqkT": np.ascontiguousarray(np.concatenate([W[qs], W[ks]], axis=0).T),
            "wvT": np.ascontiguousarray(W[vs].T),
            "bqk": np.ascontiguousarray(np.concatenate([bi[qs], bi[ks]])),
            "bv": np.ascontiguousarray(bi[vs].reshape(1, FV)),
            "woT": np.ascontiguousarray(Wo[:, g * FV:(g + 1) * FV].T),
        })
    return in_maps


def kernel(x, in_proj_w, in_proj_b, out_proj_w, out_proj_b):
    global LAST_RESULT
    nc = build_program()
    in_maps = make_in_maps(x, in_proj_w, in_proj_b, out_proj_w)
    res = bass_utils.run_bass_kernel_spmd(
        nc, in_maps, core_ids=list(range(N_CORES)), trace=TRACE,
    )
    LAST_RESULT = res
    bo = np.asarray(out_proj_b, dtype=np.float32)
    out = np.zeros((S, B, E), dtype=np.float32)
    for b in range(B):
        acc = res.results[b * 4]["y"].astype(np.float32)
        for g in range(1, 4):
            acc = acc + res.results[b * 4 + g]["y"]
        out[:, b, :] = acc + bo[None, :]
    return out
